# revision 8
# baseline (speedup 1.0000x reference)
"""Trainium2 Bass kernel for the longtail Plackett-Luce loss.

Math (per batch row b):
    sum_exp  = sum_v exp(output[b, v])
    log_pl   = output[b, target[b]] - log(sum_exp)
    exp_s[k] = mask[k] * exp(output[b, longtail[b, k]])     mask = longtail > 0
    arg[k]   = (sum_exp - exp(output[b, target[b]])) - sum_{j<k} exp_s[j]
    tail     = sum_k mask[k] * (scores[k] - log(arg[k]))
    neg_like = -(log_pl + tail) + loss_weight[target[b]]

Sharding: batch rows split across 8 NeuronCores (512 rows each), loss_weight
replicated.  Per core the 512x32000 f32 slice (65.5 MB) is streamed through
SBUF in [128, 8000] f32 tiles (4 MB, triple buffered) on the HWDGE path; the
scalar engine does exp in place with a fused row-sum (accum_out).  HBM read
bandwidth (~358 GB/s/core -> ~183 us) is the streaming roofline.

Gathers: HW indirect DMA consumes exactly ONE index per destination
partition row (wider offset APs are silently mis-lowered), so the 51
per-row scores (50 tail + target) cost 51 indirect DMAs per 128-row group.
These 204+4 gathers serialize at ~1.1 us each on the Pool/SWDGE engine and
are the critical path; everything else (index math on DVE, streaming on
HWDGE/ACT, per-group tail math, result stores last) is arranged to overlap
them completely.  The reverse cumsum uses tensor_tensor_scan.
"""

import sys

import numpy as np

sys.path.insert(0, "/opt/trn_rl_repo")

import concourse.bass as bass  # noqa: E402
import concourse.bacc as bacc  # noqa: E402
import concourse.tile as tile  # noqa: E402
from concourse import mybir  # noqa: E402
from concourse.bass_utils import run_bass_kernel_spmd  # noqa: E402

B, V, L = 4096, 32000, 50
NCORES = 8
RPC = B // NCORES  # 512 rows per core
P = 128            # SBUF partitions
G = RPC // P       # 4 row-groups per core
NCH = 2            # column chunks per row-group
C = V // NCH       # 16000 columns per chunk (8 MB f32 tiles, 64 KB descriptors)
LT = L + 1         # gathered scores per row: 50 tail + 1 target

F32 = mybir.dt.float32
I32 = mybir.dt.int32
ALU = mybir.AluOpType
ACTF = mybir.ActivationFunctionType

# Knobs test.py can flip for profiling.
TRACE = False
TRACE_KWARGS = {}
LAST_RESULTS = None

_NC_CACHE = None


def build_nc():
    nc = bacc.Bacc()
    out_t = nc.dram_tensor("output", [RPC, V], F32, kind="ExternalInput")
    tgt_t = nc.dram_tensor("tgt", [RPC, 1], I32, kind="ExternalInput")
    lt_t = nc.dram_tensor("lt", [RPC, L], I32, kind="ExternalInput")
    lw_t = nc.dram_tensor("lw", [1, V], F32, kind="ExternalInput")
    res_t = nc.dram_tensor("neg_like", [RPC, 1], F32, kind="ExternalOutput")

    out_ap = out_t[:, :]

    with tile.TileContext(nc) as tc:
        with (
            tc.tile_pool(name="stream", bufs=2) as stream,
            tc.tile_pool(name="glob", bufs=1) as glob,
            tc.tile_pool(name="pg", bufs=2) as pg,
        ):
            # rowoff[p] = p*V  (flat row offset within a group)
            rowoff = glob.tile([P, 1], I32, tag="rowoff")
            nc.gpsimd.iota(rowoff[:], pattern=[[0, 1]], base=0,
                           channel_multiplier=V)
            neg1 = glob.tile([P, L], F32, tag="neg1")
            nc.vector.memset(neg1[:], -1.0)

            # --- index prep (all groups up front, DVE only) ---
            lt_all = glob.tile([P, G * L], I32, tag="lt")
            tgt_all = glob.tile([P, G], I32, tag="tgt")
            for g in range(G):
                nc.sync.dma_start(out=lt_all[:, g * L : (g + 1) * L],
                                  in_=lt_t[g * P : (g + 1) * P, :])
                nc.sync.dma_start(out=tgt_all[:, g : g + 1],
                                  in_=tgt_t[g * P : (g + 1) * P, :])

            mask_all = glob.tile([P, G * L], F32, tag="mask")
            nc.vector.tensor_scalar(out=mask_all[:], in0=lt_all[:],
                                    scalar1=0, scalar2=None, op0=ALU.is_gt)
            # flat index into this core's output slice.  Inputs are
            # randint in [0, V) so the reference's clip is a no-op; the
            # masked (<=0) entries gather row element 0 harmlessly.
            # idx_all[p, g*LT + k] = lt[g*P+p, k] + p*V + g*P*V
            idx_all = glob.tile([P, G * LT], I32, tag="idx")
            for g in range(G):
                off = glob.tile([P, 1], I32, tag=f"off{g}")
                nc.vector.tensor_scalar(out=off[:], in0=rowoff[:],
                                        scalar1=g * P * V, scalar2=None,
                                        op0=ALU.add)
                nc.vector.tensor_tensor(
                    out=idx_all[:, g * LT : g * LT + L],
                    in0=lt_all[:, g * L : (g + 1) * L],
                    in1=off[:].to_broadcast([P, L]), op=ALU.add)
                nc.vector.tensor_tensor(
                    out=idx_all[:, g * LT + L : (g + 1) * LT],
                    in0=tgt_all[:, g : g + 1], in1=off[:], op=ALU.add)

            # --- gathers: the Pool engine does nothing but these ---
            # scores_g[p, k] = output[g*P+p, idx[p,k]]; col L = target score.
            # Round-robin across groups: consecutive Pool ops then write
            # DIFFERENT tiles, so the tile-WAW wait (full DMA round trip,
            # ~2.4 us) is already satisfied when the same tile comes up
            # again 4 ops (~4.4 us) later — no Pool stalls.
            curw_tiles = [glob.tile([P, 1], F32, tag=f"curw{g}", name=f"curw{g}")
                          for g in range(G)]
            for g in range(G):
                nc.gpsimd.indirect_dma_start(
                    out=curw_tiles[g][:], out_offset=None,
                    in_=lw_t[:, :],
                    in_offset=bass.IndirectOffsetOnAxis(
                        ap=tgt_all[:, g : g + 1], axis=1),
                )
            score_tiles = [glob.tile([P, LT], F32, tag=f"scores{g}", name=f"scores{g}")
                           for g in range(G)]
            for k in range(LT):
                for g in range(G):
                    nc.gpsimd.indirect_dma_start(
                        out=score_tiles[g][:, k : k + 1], out_offset=None,
                        in_=out_ap,
                        in_offset=bass.IndirectOffsetOnAxis(
                            ap=idx_all[:, g * LT + k : g * LT + k + 1], axis=1),
                    )

            # --- stream the 512xV slice, exp in place + row-sum chunks ---
            res = glob.tile([P, G], F32, tag="res")
            sumexp_tiles = []
            for g in range(G):
                acc = glob.tile([P, NCH], F32, tag=f"acc{g}")
                for ch in range(NCH):
                    x = stream.tile([P, C], F32, tag="x")
                    nc.sync.dma_start(
                        out=x[:],
                        in_=out_ap[g * P : (g + 1) * P, ch * C : (ch + 1) * C])
                    nc.scalar.activation(out=x[:], in_=x[:], func=ACTF.Exp,
                                         accum_out=acc[:, ch : ch + 1])
                sumexp = glob.tile([P, 1], F32, tag=f"sumexp{g}")
                sumexp_tiles.append(sumexp)
                nc.vector.tensor_reduce(out=sumexp[:], in_=acc[:],
                                        axis=mybir.AxisListType.X, op=ALU.add)

            # --- tail terms, batched by ACT table set (Exp ops, then Ln) ---
            exps_tiles = []
            for g in range(G):
                exps = glob.tile([P, LT], F32, tag=f"exps{g}")
                exps_tiles.append(exps)
                nc.scalar.activation(out=exps[:], in_=score_tiles[g][:],
                                     func=ACTF.Exp)
            arg_tiles = []
            for g in range(G):
                sumexp, exps = sumexp_tiles[g], exps_tiles[g]
                expsm = pg.tile([P, L], F32, tag="expsm")
                nc.vector.tensor_tensor(
                    out=expsm[:], in0=exps[:, 0:L],
                    in1=mask_all[:, g * L : (g + 1) * L], op=ALU.mult)
                c0 = pg.tile([P, 1], F32, tag="c0")
                nc.vector.tensor_tensor(
                    out=c0[:], in0=sumexp[:], in1=exps[:, L:LT],
                    op=ALU.subtract)
                # argbuf[:, k] = c0 - sum_{j<k} expsm[j]  (exclusive prefix);
                # col L carries sum_exp so one Ln yields log-args AND lse
                argbuf = glob.tile([P, LT], F32, tag=f"argbuf{g}")
                arg_tiles.append(argbuf)
                nc.vector.tensor_copy(out=argbuf[:, 0:1], in_=c0[:])
                nc.vector.tensor_tensor_scan(
                    out=argbuf[:, 1:LT], data0=expsm[:], data1=neg1[:],
                    initial=c0[:], op0=ALU.subtract, op1=ALU.mult)
                nc.vector.tensor_copy(out=argbuf[:, L:LT], in_=sumexp[:])
            log_tiles = []
            for g in range(G):
                logext = glob.tile([P, LT], F32, tag=f"logext{g}")
                log_tiles.append(logext)
                nc.scalar.activation(out=logext[:], in_=arg_tiles[g][:],
                                     func=ACTF.Ln)
            for g in range(G):
                logext, scores = log_tiles[g], score_tiles[g]
                scoresd = pg.tile([P, LT], F32, tag="scoresd")
                nc.vector.tensor_copy(out=scoresd[:], in_=scores[:])
                terms = pg.tile([P, L], F32, tag="terms")
                nc.vector.tensor_tensor(out=terms[:], in0=scoresd[:, 0:L],
                                        in1=logext[:, 0:L], op=ALU.subtract)
                termsm = pg.tile([P, L], F32, tag="termsm")
                nc.vector.tensor_tensor(
                    out=termsm[:], in0=terms[:],
                    in1=mask_all[:, g * L : (g + 1) * L], op=ALU.mult)
                tailsum = pg.tile([P, 1], F32, tag="tailsum")
                nc.vector.tensor_reduce(out=tailsum[:], in_=termsm[:],
                                        axis=mybir.AxisListType.X, op=ALU.add)

                # neg_like = log(sum_exp) - tgt_score - tail + cur_w
                r1 = pg.tile([P, 1], F32, tag="r1")
                nc.vector.tensor_tensor(out=r1[:], in0=logext[:, L:LT],
                                        in1=scoresd[:, L:LT],
                                        op=ALU.subtract)
                r2 = pg.tile([P, 1], F32, tag="r2")
                nc.vector.tensor_tensor(out=r2[:], in0=r1[:], in1=tailsum[:],
                                        op=ALU.subtract)
                nc.vector.tensor_tensor(out=res[:, g : g + 1], in0=r2[:],
                                        in1=curw_tiles[g][:],
                                        op=ALU.add)

            # result stores last so they never head-of-line-block the
            # HWDGE FIFO between groups
            for g in range(G):
                nc.sync.dma_start(out=res_t[g * P : (g + 1) * P, :],
                                  in_=res[:, g : g + 1])
    nc.compile()
    return nc


def kernel(output, target, longtail, loss_weight):
    global LAST_RESULTS, _NC_CACHE
    output = np.ascontiguousarray(np.asarray(output, dtype=np.float32))
    tgt = np.ascontiguousarray(np.asarray(target).astype(np.int32).reshape(B, 1))
    lt = np.ascontiguousarray(np.asarray(longtail).astype(np.int32))
    lw = np.ascontiguousarray(np.asarray(loss_weight, dtype=np.float32))

    if _NC_CACHE is None:
        _NC_CACHE = build_nc()
    nc = _NC_CACHE

    in_maps = []
    for c in range(NCORES):
        s = slice(c * RPC, (c + 1) * RPC)
        in_maps.append(
            {"output": output[s], "tgt": tgt[s], "lt": lt[s], "lw": lw.reshape(1, V)}
        )
    LAST_RESULTS = run_bass_kernel_spmd(
        nc, in_maps, core_ids=list(range(NCORES)), trace=TRACE, **TRACE_KWARGS
    )
    return np.concatenate(
        [r["neg_like"].reshape(-1) for r in LAST_RESULTS.results], axis=0
    ).astype(np.float32)


# revision 9
# speedup vs baseline: 1.3728x; 1.3728x over previous
"""Trainium2 Bass kernel for the longtail Plackett-Luce loss.

Math (per batch row b):
    sum_exp  = sum_v exp(output[b, v])
    log_pl   = output[b, target[b]] - log(sum_exp)
    exp_s[k] = mask[k] * exp(output[b, longtail[b, k]])     mask = longtail > 0
    arg[k]   = (sum_exp - exp(output[b, target[b]])) - sum_{j<k} exp_s[j]
             ( == rev_cumsum(exp_s)[k] + other   in the reference formulation )
    tail     = sum_k mask[k] * (scores[k] - log(arg[k]))
    neg_like = -(log_pl + tail) + loss_weight[target[b]]

Sharding: batch rows split across 8 NeuronCores (512 rows each), loss_weight
replicated.  Per core the 512x32000 f32 slice (65.5 MB) is streamed through
SBUF in [128, 4000] tiles; the scalar engine does exp with fused row-sum
(accum_out).  The 51 per-row gathers (50 longtail + target) are one
indirect-DMA gather per 128-row group; loss_weight[target] is a second tiny
indirect gather.  The reverse cumsum uses tensor_tensor_scan.
"""

import sys

import numpy as np

sys.path.insert(0, "/opt/trn_rl_repo")

import concourse.bass as bass  # noqa: E402
import concourse.bacc as bacc  # noqa: E402
import concourse.tile as tile  # noqa: E402
from concourse import mybir  # noqa: E402
from concourse.bass_utils import run_bass_kernel_spmd  # noqa: E402

B, V, L = 4096, 32000, 50
NCORES = 8
RPC = B // NCORES  # 512 rows per core
P = 128            # SBUF partitions
G = RPC // P       # 4 row-groups per core
NCH = 8            # column chunks per row-group
C = V // NCH       # 4000 columns per chunk

F32 = mybir.dt.float32
I32 = mybir.dt.int32
ALU = mybir.AluOpType
ACTF = mybir.ActivationFunctionType

# Knobs test.py can flip for profiling.
TRACE = False
TRACE_KWARGS = {}
LAST_RESULTS = None

_NC_CACHE = None


def build_nc():
    nc = bacc.Bacc()
    out_t = nc.dram_tensor("output", [RPC, V], F32, kind="ExternalInput")
    tgt_t = nc.dram_tensor("tgt", [RPC, 1], I32, kind="ExternalInput")
    lt_t = nc.dram_tensor("lt", [RPC, L], I32, kind="ExternalInput")
    lw_t = nc.dram_tensor("lw", [1, V], F32, kind="ExternalInput")
    res_t = nc.dram_tensor("neg_like", [RPC, 1], F32, kind="ExternalOutput")

    out_ap = out_t[:, :]

    with tile.TileContext(nc) as tc:
        with (
            tc.tile_pool(name="stream", bufs=4) as stream,
            tc.tile_pool(name="scratch", bufs=2) as scratch,
            tc.tile_pool(name="small", bufs=4) as small,
            tc.tile_pool(name="consts", bufs=1) as consts,
        ):
            # rowoff[p] = p * V  (partition -> flat row offset within a group)
            rowoff = consts.tile([P, 1], I32)
            nc.gpsimd.iota(rowoff[:], pattern=[[0, 1]], base=0, channel_multiplier=V)
            neg1 = consts.tile([P, L], F32)
            nc.vector.memset(neg1[:], -1.0)

            r3_tiles = []
            for g in range(G):
                r0 = g * P

                # --- index prep + gathers ---
                lt_sb = small.tile([P, L], I32, tag="lt")
                nc.sync.dma_start(out=lt_sb[:], in_=lt_t[r0 : r0 + P, :])
                tgt_sb = small.tile([P, 1], I32, tag="tgt")
                nc.sync.dma_start(out=tgt_sb[:], in_=tgt_t[r0 : r0 + P, :])

                mask = small.tile([P, L], F32, tag="mask")
                nc.vector.tensor_scalar(
                    out=mask[:], in0=lt_sb[:], scalar1=0, scalar2=None,
                    op0=ALU.is_gt,
                )
                # clip(longtail, 0, V-1), matching the reference
                idx_clip = small.tile([P, L], I32, tag="idxclip")
                nc.gpsimd.tensor_scalar(
                    out=idx_clip[:], in0=lt_sb[:], scalar1=0, scalar2=V - 1,
                    op0=ALU.max, op1=ALU.min,
                )
                idx_all = small.tile([P, L + 1], I32, tag="idx")
                nc.gpsimd.tensor_tensor(
                    out=idx_all[:, 0:L], in0=idx_clip[:],
                    in1=rowoff[:].to_broadcast([P, L]), op=ALU.add,
                )
                nc.gpsimd.tensor_tensor(
                    out=idx_all[:, L : L + 1], in0=tgt_sb[:],
                    in1=rowoff[:], op=ALU.add,
                )

                # scores[p, k] = output[r0 + p, idx[p, k]]; col L = target score
                # HW indirect DMA consumes ONE index per dest partition-row,
                # so issue one [128,1] gather per tail position.
                scores = small.tile([P, L + 1], F32, tag="scores")
                for k in range(L + 1):
                    nc.gpsimd.indirect_dma_start(
                        out=scores[:, k : k + 1], out_offset=None,
                        in_=out_ap,
                        in_offset=bass.IndirectOffsetOnAxis(
                            ap=idx_all[:, k : k + 1], axis=1
                        ),
                        element_offset=r0 * V,
                    )
                curw = small.tile([P, 1], F32, tag="curw")
                nc.gpsimd.indirect_dma_start(
                    out=curw[:], out_offset=None,
                    in_=lw_t[:, :],
                    in_offset=bass.IndirectOffsetOnAxis(ap=tgt_sb[:], axis=1),
                )
                # DVE-side copies: later DVE consumers then have same-engine
                # deps (the DVE TensorTensor struct allows only ONE sync wait)
                scoresd = small.tile([P, L + 1], F32, tag="scoresd")
                nc.vector.tensor_copy(out=scoresd[:], in_=scores[:])

                # --- stream the row-group, exp + accumulate row sums ---
                acc = small.tile([P, NCH], F32, tag="acc")
                for ch in range(NCH):
                    x = stream.tile([P, C], F32, tag="x")
                    nc.sync.dma_start(
                        out=x[:], in_=out_ap[r0 : r0 + P, ch * C : (ch + 1) * C]
                    )
                    e = scratch.tile([P, C], F32, tag="e")
                    nc.scalar.activation(
                        out=e[:], in_=x[:], func=ACTF.Exp,
                        accum_out=acc[:, ch : ch + 1],
                    )
                sumexp = small.tile([P, 1], F32, tag="sumexp")
                nc.vector.tensor_reduce(
                    out=sumexp[:], in_=acc[:], axis=mybir.AxisListType.X, op=ALU.add
                )

                # --- tail term ---
                expt = small.tile([P, 1], F32, tag="expt")
                nc.scalar.activation(out=expt[:], in_=scores[:, L : L + 1], func=ACTF.Exp)
                exps = small.tile([P, L], F32, tag="exps")
                nc.scalar.activation(out=exps[:], in_=scores[:, 0:L], func=ACTF.Exp)
                expsm = small.tile([P, L], F32, tag="expsm")
                nc.vector.tensor_tensor(out=expsm[:], in0=exps[:], in1=mask[:], op=ALU.mult)
                c0 = small.tile([P, 1], F32, tag="c0")
                nc.vector.tensor_tensor(out=c0[:], in0=sumexp[:], in1=expt[:], op=ALU.subtract)

                # argbuf[:, k] = c0 - sum_{j<k} expsm[j]   (exclusive prefix)
                # scan: state = (expsm[t] - state) * (-1)  => state -= expsm[t]
                argbuf = small.tile([P, L + 1], F32, tag="argbuf")
                nc.vector.tensor_copy(out=argbuf[:, 0:1], in_=c0[:])
                nc.vector.tensor_tensor_scan(
                    out=argbuf[:, 1 : L + 1], data0=expsm[:], data1=neg1[:],
                    initial=c0[:], op0=ALU.subtract, op1=ALU.mult,
                )
                logarg = small.tile([P, L], F32, tag="logarg")
                nc.scalar.activation(out=logarg[:], in_=argbuf[:, 0:L], func=ACTF.Ln)
                terms = small.tile([P, L], F32, tag="terms")
                nc.vector.tensor_tensor(out=terms[:], in0=scoresd[:, 0:L], in1=logarg[:], op=ALU.subtract)
                termsm = small.tile([P, L], F32, tag="termsm")
                nc.vector.tensor_tensor(out=termsm[:], in0=terms[:], in1=mask[:], op=ALU.mult)
                tailsum = small.tile([P, 1], F32, tag="tailsum")
                nc.vector.tensor_reduce(
                    out=tailsum[:], in_=termsm[:], axis=mybir.AxisListType.X, op=ALU.add
                )

                # neg_like = log(sum_exp) - target_score - tail + cur_w
                lse = small.tile([P, 1], F32, tag="lse")
                nc.scalar.activation(out=lse[:], in_=sumexp[:], func=ACTF.Ln)
                r1 = small.tile([P, 1], F32, tag="r1")
                nc.vector.tensor_tensor(out=r1[:], in0=lse[:], in1=scoresd[:, L : L + 1], op=ALU.subtract)
                r2 = small.tile([P, 1], F32, tag="r2")
                nc.vector.tensor_tensor(out=r2[:], in0=r1[:], in1=tailsum[:], op=ALU.subtract)
                r3 = small.tile([P, 1], F32, tag="r3")
                nc.vector.tensor_tensor(out=r3[:], in0=r2[:], in1=curw[:], op=ALU.add)
                r3_tiles.append(r3)
            # stores last: a store stuck waiting on tail math would
            # head-of-line-block the next group's stream DMAs in the
            # HWDGE FIFO
            for g, r3 in enumerate(r3_tiles):
                nc.sync.dma_start(out=res_t[g * P : (g + 1) * P, :], in_=r3[:])
    nc.compile()
    return nc


def kernel(output, target, longtail, loss_weight):
    global LAST_RESULTS, _NC_CACHE
    output = np.ascontiguousarray(np.asarray(output, dtype=np.float32))
    tgt = np.ascontiguousarray(np.asarray(target).astype(np.int32).reshape(B, 1))
    lt = np.ascontiguousarray(np.asarray(longtail).astype(np.int32))
    lw = np.ascontiguousarray(np.asarray(loss_weight, dtype=np.float32))

    if _NC_CACHE is None:
        _NC_CACHE = build_nc()
    nc = _NC_CACHE

    in_maps = []
    for c in range(NCORES):
        s = slice(c * RPC, (c + 1) * RPC)
        in_maps.append(
            {"output": output[s], "tgt": tgt[s], "lt": lt[s], "lw": lw.reshape(1, V)}
        )
    LAST_RESULTS = run_bass_kernel_spmd(
        nc, in_maps, core_ids=list(range(NCORES)), trace=TRACE, **TRACE_KWARGS
    )
    return np.concatenate(
        [r["neg_like"].reshape(-1) for r in LAST_RESULTS.results], axis=0
    ).astype(np.float32)

